# revision 70
# baseline (speedup 1.0000x reference)
"""Trainium2 Bass kernel: 16-head causal self-attention block (QKV proj ->
causal MHA -> output proj), tensor-parallel over heads across 8 NeuronCores.

Contract: kernel(**inputs) takes FULL unsharded inputs
  x      [2, 2048, 1024] f32
  w_qkv  [1024, 3072] f32, b_qkv [3072] f32
  w_proj [1024, 1024] f32, b_proj [1024] f32
and returns the FULL output [2, 2048, 1024] f32.

Sharding: head-parallel. Core c owns global heads (2c, 2c+1):
  - column-parallel QKV (each core takes its 128 q/k/v feature columns)
  - full causal attention for its 2 heads (both batches)
  - row-parallel output projection -> partial [4096, 1024] sums
  - host reduces the 8 partials and adds b_proj.

v2 dataflow (vs v1): software-pipelined emission interleaves QKV of
super-tile s with attention of super s-1 so the PE never starves while
ScalarE runs softmax exp (ScalarE does ONLY exp + a few psum drains; no
Ln -> no act-table thrash). q/k/P/v all fp16 on the PE. v is computed in
natural [token, feat] orientation directly (xT stationary), killing the
PE transposes; v bias folds in at the PSUM drain via a precomputed
broadcast tile. x is pre-transposed on the host so all x loads are plain
contiguous DMAs. 1/Z via DVE reciprocal + rank-1 broadcast matmul.
"""

import numpy as np
from contextlib import ExitStack

import concourse.bass as bass
import concourse.tile as tile
from concourse import bacc, mybir
from concourse.bass_utils import run_bass_kernel_spmd
from concourse.masks import make_upper_triangular

F32 = mybir.dt.float32
F32R = mybir.dt.float32r
F16 = mybir.dt.float16
AF = mybir.ActivationFunctionType

N_CORES = 8
B, T, E, H, D = 2, 2048, 1024, 16, 64
TOK = B * T          # 4096 tokens
P = 128              # partitions
SUPER = 512          # tokens per QKV super-tile
NS = TOK // SUPER    # 8 super-tiles
KCH = E // P         # 8 contraction chunks
QTL = 512            # attention q-tile width
NQT = T // QTL       # 4 q-tiles per batch
KBL = 128            # attention k-block height
VAW = 256            # v_aug cols per token tile: 2 heads x (64 v + 64 ones);
                     # the 64 replicated ones-columns make the AV matmul emit
                     # the softmax denominator Z broadcast on partitions 64-127


def r(ap):
    return ap.bitcast(F32R)


def _emit(nc, tc, ctx):
    # weights arrive host-relayouted: [P, E] chunk-major, one contiguous DMA
    xT_h = nc.declare_dram_parameter("xT", [E, TOK], F16, isOutput=False)
    wq_h = nc.declare_dram_parameter("wq", [P, E], F16, isOutput=False)
    wk_h = nc.declare_dram_parameter("wk", [P, E], F16, isOutput=False)
    wv_h = nc.declare_dram_parameter("wv", [P, E], F16, isOutput=False)
    bqk_h = nc.declare_dram_parameter("bqk", [P, 2], F32, isOutput=False)
    bv_h = nc.declare_dram_parameter("bv", [1, P], F16, isOutput=False)
    wp_h = nc.declare_dram_parameter("wp", [P, E], F16, isOutput=False)
    out_h = nc.declare_dram_parameter("out", [TOK, E], F32, isOutput=True)

    outr = out_h[:].rearrange("(n p) e -> n p e", p=P)  # [32, 128, 1024]

    # ---------------- persistent tiles ----------------
    const = ctx.enter_context(tc.tile_pool(name="const", bufs=1))
    mask_tri = const.tile([P, P], F16)  # mask[p, f] = 1.0 iff p <= f
    make_upper_triangular(nc, mask_tri[:], val=1.0, diag=True)
    ones1h = const.tile([1, P], F16)
    nc.vector.memset(ones1h[:], 1.0)
    e33 = const.tile([33, P], F16)
    nc.vector.memset(e33[:], 0.0)
    nc.vector.memset(e33[0:1, 0:D], 1.0)
    nc.vector.memset(e33[32:33, D:2 * D], 1.0)

    # weight/bias loads go on the Activation queue (idle at startup) so the
    # x loads on the sync queue aren't delayed behind them
    bqk_sb = const.tile([P, 2], F32)
    bvr = const.tile([1, P], F16)
    nc.scalar.dma_start(bqk_sb[:], bqk_h[:])
    nc.scalar.dma_start(bvr[:], bv_h[:])
    bq_sb = bqk_sb[:, 0:1]
    bk_sb = bqk_sb[:, 1:2]
    wq_sb = const.tile([P, E], F16)
    wk_sb = const.tile([P, E], F16)
    wv_sb = const.tile([P, E], F16)
    wp_sb = const.tile([P, E], F16)
    for wsb, wh in ((wq_sb, wq_h), (wk_sb, wk_h), (wv_sb, wv_h),
                    (wp_sb, wp_h)):
        nc.scalar.dma_start(wsb[:], wh[:])
    bvb = const.tile([P, P], F32)  # v bias broadcast to all 128 partitions

    persist = ctx.enter_context(tc.tile_pool(name="persist", bufs=1))
    zrowp = persist.tile([33, QTL], F16)
    nc.vector.memset(zrowp[:], 0.0)

    with ExitStack() as ph:
        xpool = ph.enter_context(tc.tile_pool(name="xp", bufs=4))
        ptpool = ph.enter_context(tc.tile_pool(name="pTp", bufs=8))
        zbpool = ph.enter_context(tc.tile_pool(name="zbp", bufs=2))
        opool = ph.enter_context(tc.tile_pool(name="op", bufs=4))
        poolQ = ph.enter_context(tc.tile_pool(name="poolQ", bufs=2, space="PSUM"))
        poolS = ph.enter_context(tc.tile_pool(name="poolS", bufs=2, space="PSUM"))
        poolY = ph.enter_context(tc.tile_pool(name="poolY", bufs=2, space="PSUM"))

        # v bias broadcast: bvb[p, j] = bv[j] via rank-1 matmul of ones x bv
        pb = poolQ.tile([P, P], F32, tag="q", name="pb")
        nc.tensor.matmul(pb[:], lhsT=ones1h[:], rhs=bvr[:], start=True,
                         stop=True)
        nc.vector.tensor_copy(bvb[:], pb[:])

        yts = [persist.tile([P, QTL], F16, tag=f"yt{n}", name="yt")
               for n in range(NS)]
        projq = []
        qTs, kTs, vas = [], [], []
        for s in range(NS):
            qTs.append(persist.tile([P, SUPER], F16, tag=f"qT{s}", name="qTt"))
            kTs.append(persist.tile([P, SUPER], F16, tag=f"kT{s}", name="kTt"))
            vat = persist.tile([P, 4 * VAW], F16, tag=f"va{s}", name="vat")
            vas.append(vat)
            # whole tile starts as ones; the per-super v drains overwrite the
            # v columns, leaving the replicated ones-columns (64-127 of each
            # head group) that make the AV matmul emit Z on partitions 64-127
            nc.gpsimd.memset(vat[:], 1.0)

        xtiles = {}
        xT_cm = xT_h[:].rearrange("(c p) tok -> p c tok", p=P)  # [128,8,4096]

        def issue_x(s):
            # one strided DMA per super-tile; chunk ch lands contiguous at
            # cols [ch*512, (ch+1)*512)
            xt = xpool.tile([P, KCH * SUPER], F16, tag="xT", name="xTt")
            nc.sync.dma_start(
                xt[:].rearrange("p (c t) -> p c t", c=KCH),
                xT_cm[:, :, s * SUPER:(s + 1) * SUPER])
            xtiles[s] = [xt[:, ch * SUPER:(ch + 1) * SUPER]
                         for ch in range(KCH)]

        issue_x(0)
        issue_x(1)
        issue_x(2)

        def qkv_units(s):
            xt = xtiles[s]
            if s + 3 < NS:
                issue_x(s + 3)
            yield
            pfq = poolQ.tile([P, SUPER], F32, tag="q", name="pfq")
            pfk = poolQ.tile([P, SUPER], F32, tag="q", name="pfk")
            for ch in range(KCH):
                nc.tensor.matmul(
                    pfq[:], lhsT=wq_sb[:, ch * P:(ch + 1) * P],
                    rhs=xt[ch][:], start=(ch == 0), stop=(ch == KCH - 1))
                nc.tensor.matmul(
                    pfk[:], lhsT=wk_sb[:, ch * P:(ch + 1) * P],
                    rhs=xt[ch][:], start=(ch == 0), stop=(ch == KCH - 1))
                yield
            nc.vector.tensor_scalar_add(qTs[s][:], pfq[:], bq_sb)
            nc.vector.tensor_scalar_add(kTs[s][:], pfk[:], bk_sb)
            yield
            vps = poolQ.tile([P, SUPER], F32, tag="q", name="vps")
            bvb2 = bvb[:].rearrange("p (h d) -> p h d", h=2)
            for tt in range(4):
                for ch in range(KCH):
                    nc.tensor.matmul(
                        vps[:, tt * P:(tt + 1) * P],
                        lhsT=xt[ch][:, tt * P:(tt + 1) * P],
                        rhs=wv_sb[:, ch * P:(ch + 1) * P],
                        start=(ch == 0), stop=(ch == KCH - 1))
                dst = vas[s][:, tt * VAW:(tt + 1) * VAW].rearrange(
                    "p (h x) -> p h x", x=2 * D)[:, :, 0:D]
                src = vps[:, tt * P:(tt + 1) * P].rearrange(
                    "p (h d) -> p h d", h=2)
                nc.vector.tensor_add(dst, src, bvb2)
                yield

        # Attention is split into two decoupled emission streams per q-tile:
        #   A (sexp_units): S matmuls -> exp -> mask, paced by poolS/ptpool
        #   B (av_units):   AV accumulation -> normalize -> proj -> out
        # A-streams of several q-tiles interleave freely (they never touch
        # pys), while B-streams stay strictly sequential, so a new tile's
        # S/exp runs ahead without the AV head-of-line deadlock on pys banks.
        pts = {}
        aprog = {}
        bprog = {}

        def sexp_units(b, qi):
            nkb = 4 * qi + 4   # k blocks of 128 covering [0, (qi+1)*512)
            sq = 4 * b + qi
            key = (b, qi)

            def emit_S(kb):
                c0 = max(0, kb * KBL - qi * QTL)
                sk, kc = 4 * b + kb // 4, (kb % 4) * KBL
                ps = poolS.tile([P, 2 * QTL], F32, tag="s", name="ps")
                for h in range(2):
                    nc.tensor.matmul(
                        ps[:, h * QTL + c0:(h + 1) * QTL],
                        lhsT=kTs[sk][64 * h:64 * h + 64, kc:kc + KBL],
                        rhs=qTs[sq][64 * h:64 * h + 64, c0:QTL],
                        start=True, stop=True)
                return ps, c0

            cur = emit_S(0)
            for kb in range(nkb):
                ps, c0 = cur
                if kb + 1 < nkb:
                    cur = emit_S(kb + 1)
                pt = ptpool.tile([P, 2 * QTL], F16, tag="pT", name="pt")
                if c0 == 0:
                    nc.scalar.activation(pt[:], ps[:], AF.Exp, scale=0.125)
                else:
                    src = ps[:].rearrange("p (h q) -> p h q", h=2)[:, :, c0:]
                    dst = pt[:].rearrange("p (h q) -> p h q", h=2)[:, :, c0:]
                    nc.scalar.activation(dst, src, AF.Exp, scale=0.125)
                if kb * KBL >= qi * QTL:  # diagonal block: causal triangle
                    sl = pt[:].rearrange("p (h q) -> p h q",
                                         h=2)[:, :, c0:c0 + P]
                    m3 = mask_tri[:].rearrange(
                        "p (u f) -> p u f", u=1).broadcast_to([P, 2, P])
                    nc.vector.tensor_mul(sl, sl, m3)
                pts[(b, qi, kb)] = (pt, c0)
                aprog[key] += 1
                yield

        def av_units(b, qi):
            nkb = 4 * qi + 4
            sq = 4 * b + qi
            key = (b, qi)
            # the last q-tile takes its accumulators from poolQ (idle once
            # QKV is done) so its AVs overlap the previous q-tile's tail
            ypool, ytag = (poolQ, "q") if sq == NS - 1 else (poolY, "y")
            pys = [ypool.tile([P, QTL], F32, tag=ytag, name=f"py{h}")
                   for h in range(2)]
            for kb in range(nkb):
                pt, c0 = pts.pop((b, qi, kb))
                vo = (kb % 4) * VAW
                sk = 4 * b + kb // 4
                for h in range(2):
                    nc.tensor.matmul(
                        pys[h][:, c0:QTL],
                        lhsT=vas[sk][:, vo + 2 * D * h:vo + 2 * D * h + 2 * D],
                        rhs=pt[:, h * QTL + c0:(h + 1) * QTL],
                        start=(kb == 0), stop=(kb == nkb - 1))
                bprog[key] += 1
                yield
            # normalize: y * (1/Z): broadcast Z rows to 128 partitions with a
            # rank-1 matmul, then reciprocal + scale in 128-col chunks
            nc.vector.tensor_copy(zrowp[0:1, :], pys[0][D:D + 1, :])
            nc.vector.tensor_copy(zrowp[32:33, :], pys[1][D:D + 1, :])
            pz = poolS.tile([P, QTL], F32, tag="s", name="pz")
            nc.tensor.matmul(pz[:], lhsT=e33[:], rhs=zrowp[:],
                             start=True, stop=True)
            yt = yts[sq]
            yield
            for tt4 in range(4):
                cs = slice(tt4 * P, (tt4 + 1) * P)
                zbr = zbpool.tile([P, P], F32, tag="zb", name="zbr")
                nc.vector.reciprocal(zbr[:], pz[:, cs])
                for h in range(2):
                    nc.vector.tensor_mul(
                        yt[64 * h:64 * h + D, cs],
                        pys[h][0:D, cs],
                        zbr[64 * h:64 * h + D, :],
                    )
                if tt4 % 2 == 1:
                    yield
            for tt4 in range(4):
                cs = slice(tt4 * P, (tt4 + 1) * P)
                pos = poolS.tile([P, E], F32, tag="s", name="po")
                for oc in range(2):
                    nc.tensor.matmul(
                        pos[:, oc * 512:(oc + 1) * 512],
                        lhsT=yt[:, cs],
                        rhs=wp_sb[:, oc * 512:(oc + 1) * 512],
                        start=True, stop=True)
                ti = (b * T + qi * QTL) // P + tt4
                ot = opool.tile([P, E], F32, tag="ot", name="ot")
                if tt4 % 2 == 0:
                    nc.vector.tensor_copy(ot[:], pos[:])
                else:
                    nc.scalar.activation(ot[:], pos[:], AF.Copy)
                nc.gpsimd.dma_start(outr[ti], ot[:])
                yield

        # ---- software-pipelined emission driver ----
        # Invariant: A never emits more than 4 unconsumed blocks (8 pt bufs
        # minus slack), so an exp can never block on a pt slot whose AV sits
        # behind it - the resource cycle that deadlocks poolS/pz otherwise.
        from collections import deque
        apending = []          # list of (gen, key)
        bqueue = deque()
        arr = [0]
        A_BOUND = 4

        def a_ahead():
            return sum(aprog.values()) - sum(bprog.values())

        def _advance(g):
            try:
                next(g)
                return True
            except StopIteration:
                return False

        def pump_a(n, key=None):
            pumped = 0
            while pumped < n and apending:
                if key is not None:
                    cand = [e for e in apending if e[1] == key]
                    if not cand:
                        return pumped
                    e = cand[0]
                else:
                    e = apending[arr[0] % len(apending)]
                if _advance(e[0]):
                    pumped += 1
                    arr[0] += 1
                else:
                    apending.remove(e)
            return pumped

        def pump_b(n):
            # only the head B-stream advances; its AV(kb) may only emit once
            # A emitted exp(kb) (pts handoff)
            pumped = 0
            while pumped < n and bqueue:
                g, key, nkb = bqueue[0]
                if bprog[key] < nkb and bprog[key] >= aprog[key]:
                    # head starved of its A: advance that A-gen directly
                    if a_ahead() >= A_BOUND + 2 or pump_a(1, key=key) == 0:
                        return pumped
                    continue
                if _advance(g):
                    pumped += 1
                else:
                    bqueue.popleft()
            return pumped

        def pump_mix(na, nb):
            pump_b(nb)
            while na > 0 and apending and a_ahead() < A_BOUND:
                if pump_a(1) == 0:
                    break
                na -= 1
                pump_b(1)

        QU = 14  # units per qkv super (1 issue + 8 chunks + 1 drain + 4 v)
        for s in range(NS):
            # pace the A-backlog over ~2 supers so exp work accumulates
            # toward the exp-paced back half; keep B close behind A
            start_backlog = sum(n2 - aprog[k2] for _, k2, n2 in bqueue)
            b, qi = divmod(s, NQT)
            done, k = 0, 0
            for u in qkv_units(s):
                k += 1
                want = (start_backlog * k) // (2 * QU)
                if want > done:
                    pump_mix(want - done, 1)
                    done = want
                pump_b(2)
            key = (b, qi)
            aprog[key] = 0
            bprog[key] = 0
            apending.append((sexp_units(b, qi), key))
            bqueue.append((av_units(b, qi), key, 4 * qi + 4))
        while apending or bqueue:
            got = pump_b(2)
            while apending and a_ahead() < A_BOUND and pump_a(1):
                got += 1
                pump_b(1)
            if got == 0:
                if pump_b(1) == 0 and (not apending or pump_a(1) == 0):
                    break


_NC_CACHE = None


def _build():
    global _NC_CACHE
    if _NC_CACHE is None:
        nc = bacc.Bacc("TRN2", target_bir_lowering=False, debug=False)
        with tile.TileContext(nc) as tc:
            with ExitStack() as ctx:
                _emit(nc, tc, ctx)
        nc.compile()
        _NC_CACHE = nc
    return _NC_CACHE


def make_in_maps(x, w_qkv, b_qkv, w_proj):
    x2 = np.asarray(x, dtype=np.float32).reshape(TOK, E).astype(np.float16)
    xT = np.ascontiguousarray(x2.T)  # [E, TOK] feature-major
    w_qkv = np.asarray(w_qkv, dtype=np.float32)
    b_qkv = np.asarray(b_qkv, dtype=np.float32)
    w_proj = np.asarray(w_proj, dtype=np.float32)
    def cm(w):  # [E, P] slice -> [P, E] chunk-major fp16
        return np.ascontiguousarray(
            w.astype(np.float16).reshape(KCH, P, P).transpose(1, 0, 2)
            .reshape(P, E))

    in_maps = []
    for c in range(N_CORES):
        lo = P * c
        in_maps.append({
            "xT": xT,
            "wq": cm(w_qkv[:, lo:lo + P]),
            "wk": cm(w_qkv[:, E + lo:E + lo + P]),
            "wv": cm(w_qkv[:, 2 * E + lo:2 * E + lo + P]),
            "bqk": np.ascontiguousarray(
                np.stack([b_qkv[lo:lo + P],
                          b_qkv[E + lo:E + lo + P]], axis=1)
                .astype(np.float32)),
            "bv": np.ascontiguousarray(
                b_qkv[2 * E + lo:2 * E + lo + P].astype(np.float16)
                .reshape(1, P)),
            "wp": np.ascontiguousarray(w_proj[lo:lo + P, :].astype(np.float16)),
        })
    return in_maps


def run_sharded(inputs, trace=False, **kw):
    nc = _build()
    in_maps = make_in_maps(inputs["x"], inputs["w_qkv"], inputs["b_qkv"],
                           inputs["w_proj"])
    res = run_bass_kernel_spmd(nc, in_maps, list(range(N_CORES)), trace=trace,
                               **kw)
    partial = np.zeros((TOK, E), dtype=np.float32)
    for i in range(N_CORES):
        partial += res.results[i]["out"]
    out = partial + np.asarray(inputs["b_proj"], dtype=np.float32)[None, :]
    return out.reshape(B, T, E), res


def kernel(**inputs) -> np.ndarray:
    out, _ = run_sharded(inputs, trace=False)
    return out


# revision 73
# speedup vs baseline: 1.0077x; 1.0077x over previous
"""Trainium2 Bass kernel: 16-head causal self-attention block (QKV proj ->
causal MHA -> output proj), tensor-parallel over heads across 8 NeuronCores.

Contract: kernel(**inputs) takes FULL unsharded inputs
  x      [2, 2048, 1024] f32
  w_qkv  [1024, 3072] f32, b_qkv [3072] f32
  w_proj [1024, 1024] f32, b_proj [1024] f32
and returns the FULL output [2, 2048, 1024] f32.

Sharding: head-parallel. Core c owns global heads (2c, 2c+1):
  - column-parallel QKV (each core takes its 128 q/k/v feature columns)
  - full causal attention for its 2 heads (both batches)
  - row-parallel output projection -> partial [4096, 1024] sums
  - host reduces the 8 partials and adds b_proj.

v2 dataflow (vs v1): software-pipelined emission interleaves QKV of
super-tile s with attention of super s-1 so the PE never starves while
ScalarE runs softmax exp (ScalarE does ONLY exp + a few psum drains; no
Ln -> no act-table thrash). q/k/P/v all fp16 on the PE. v is computed in
natural [token, feat] orientation directly (xT stationary), killing the
PE transposes; v bias folds in at the PSUM drain via a precomputed
broadcast tile. x is pre-transposed on the host so all x loads are plain
contiguous DMAs. 1/Z via DVE reciprocal + rank-1 broadcast matmul.
"""

import numpy as np
from contextlib import ExitStack

import concourse.bass as bass
import concourse.tile as tile
from concourse import bacc, mybir
from concourse.bass_utils import run_bass_kernel_spmd
from concourse.masks import make_upper_triangular

F32 = mybir.dt.float32
F32R = mybir.dt.float32r
F16 = mybir.dt.float16
AF = mybir.ActivationFunctionType

N_CORES = 8
B, T, E, H, D = 2, 2048, 1024, 16, 64
TOK = B * T          # 4096 tokens
P = 128              # partitions
SUPER = 512          # tokens per QKV super-tile
NS = TOK // SUPER    # 8 super-tiles
KCH = E // P         # 8 contraction chunks
QTL = 512            # attention q-tile width
NQT = T // QTL       # 4 q-tiles per batch
KBL = 128            # attention k-block height
VAW = 256            # v_aug cols per token tile: 2 heads x (64 v + 64 ones);
                     # the 64 replicated ones-columns make the AV matmul emit
                     # the softmax denominator Z broadcast on partitions 64-127


def r(ap):
    return ap.bitcast(F32R)


def _emit(nc, tc, ctx):
    # weights arrive host-relayouted: [P, E] chunk-major, one contiguous DMA
    xT_h = nc.declare_dram_parameter("xT", [E, TOK], F16, isOutput=False)
    wq_h = nc.declare_dram_parameter("wq", [P, E], F16, isOutput=False)
    wk_h = nc.declare_dram_parameter("wk", [P, E], F16, isOutput=False)
    wv_h = nc.declare_dram_parameter("wv", [P, E], F16, isOutput=False)
    bqk_h = nc.declare_dram_parameter("bqk", [P, 2], F32, isOutput=False)
    bv_h = nc.declare_dram_parameter("bv", [1, P], F16, isOutput=False)
    wp_h = nc.declare_dram_parameter("wp", [P, E], F16, isOutput=False)
    out_h = nc.declare_dram_parameter("out", [TOK, E], F32, isOutput=True)

    outr = out_h[:].rearrange("(n p) e -> n p e", p=P)  # [32, 128, 1024]

    # ---------------- persistent tiles ----------------
    const = ctx.enter_context(tc.tile_pool(name="const", bufs=1))
    mask_tri = const.tile([P, P], F16)  # mask[p, f] = 1.0 iff p <= f
    make_upper_triangular(nc, mask_tri[:], val=1.0, diag=True)
    ones1h = const.tile([1, P], F16)
    nc.vector.memset(ones1h[:], 1.0)
    e33 = const.tile([33, P], F16)
    nc.vector.memset(e33[:], 0.0)
    nc.vector.memset(e33[0:1, 0:D], 1.0)
    nc.vector.memset(e33[32:33, D:2 * D], 1.0)

    # weight/bias loads go on the Activation queue (idle at startup) so the
    # x loads on the sync queue aren't delayed behind them
    bqk_sb = const.tile([P, 2], F32)
    bvr = const.tile([1, P], F16)
    nc.scalar.dma_start(bqk_sb[:], bqk_h[:])
    nc.scalar.dma_start(bvr[:], bv_h[:])
    bq_sb = bqk_sb[:, 0:1]
    bk_sb = bqk_sb[:, 1:2]
    wq_sb = const.tile([P, E], F16)
    wk_sb = const.tile([P, E], F16)
    wv_sb = const.tile([P, E], F16)
    wp_sb = const.tile([P, E], F16)
    for wsb, wh in ((wq_sb, wq_h), (wk_sb, wk_h), (wv_sb, wv_h),
                    (wp_sb, wp_h)):
        nc.scalar.dma_start(wsb[:], wh[:])
    bvb = const.tile([P, P], F32)  # v bias broadcast to all 128 partitions

    persist = ctx.enter_context(tc.tile_pool(name="persist", bufs=1))
    zrowp = persist.tile([33, QTL], F16)
    nc.vector.memset(zrowp[:], 0.0)

    with ExitStack() as ph:
        xpool = ph.enter_context(tc.tile_pool(name="xp", bufs=4))
        ptpool = ph.enter_context(tc.tile_pool(name="pTp", bufs=12))
        zbpool = ph.enter_context(tc.tile_pool(name="zbp", bufs=2))
        opool = ph.enter_context(tc.tile_pool(name="op", bufs=4))
        poolQ = ph.enter_context(tc.tile_pool(name="poolQ", bufs=2, space="PSUM"))
        poolS = ph.enter_context(tc.tile_pool(name="poolS", bufs=2, space="PSUM"))
        poolY = ph.enter_context(tc.tile_pool(name="poolY", bufs=2, space="PSUM"))

        # v bias broadcast: bvb[p, j] = bv[j] via rank-1 matmul of ones x bv
        pb = poolQ.tile([P, P], F32, tag="q", name="pb")
        nc.tensor.matmul(pb[:], lhsT=ones1h[:], rhs=bvr[:], start=True,
                         stop=True)
        nc.vector.tensor_copy(bvb[:], pb[:])

        yts = [persist.tile([P, QTL], F16, tag=f"yt{n}", name="yt")
               for n in range(NS)]
        projq = []
        qTs, kTs, vas = [], [], []
        for s in range(NS):
            qTs.append(persist.tile([P, SUPER], F16, tag=f"qT{s}", name="qTt"))
            kTs.append(persist.tile([P, SUPER], F16, tag=f"kT{s}", name="kTt"))
            vat = persist.tile([P, 4 * VAW], F16, tag=f"va{s}", name="vat")
            vas.append(vat)
            # whole tile starts as ones; the per-super v drains overwrite the
            # v columns, leaving the replicated ones-columns (64-127 of each
            # head group) that make the AV matmul emit Z on partitions 64-127
            nc.gpsimd.memset(vat[:], 1.0)

        xtiles = {}
        xT_cm = xT_h[:].rearrange("(c p) tok -> p c tok", p=P)  # [128,8,4096]

        def issue_x(s):
            # one strided DMA per super-tile; chunk ch lands contiguous at
            # cols [ch*512, (ch+1)*512)
            xt = xpool.tile([P, KCH * SUPER], F16, tag="xT", name="xTt")
            nc.sync.dma_start(
                xt[:].rearrange("p (c t) -> p c t", c=KCH),
                xT_cm[:, :, s * SUPER:(s + 1) * SUPER])
            xtiles[s] = [xt[:, ch * SUPER:(ch + 1) * SUPER]
                         for ch in range(KCH)]

        issue_x(0)
        issue_x(1)
        issue_x(2)

        def qkv_units(s):
            xt = xtiles[s]
            if s + 3 < NS:
                issue_x(s + 3)
            yield
            pfq = poolQ.tile([P, SUPER], F32, tag="q", name="pfq")
            pfk = poolQ.tile([P, SUPER], F32, tag="q", name="pfk")
            for ch in range(KCH):
                nc.tensor.matmul(
                    pfq[:], lhsT=wq_sb[:, ch * P:(ch + 1) * P],
                    rhs=xt[ch][:], start=(ch == 0), stop=(ch == KCH - 1))
                nc.tensor.matmul(
                    pfk[:], lhsT=wk_sb[:, ch * P:(ch + 1) * P],
                    rhs=xt[ch][:], start=(ch == 0), stop=(ch == KCH - 1))
                yield
            nc.vector.tensor_scalar_add(qTs[s][:], pfq[:], bq_sb)
            nc.vector.tensor_scalar_add(kTs[s][:], pfk[:], bk_sb)
            yield
            vps = poolQ.tile([P, SUPER], F32, tag="q", name="vps")
            bvb2 = bvb[:].rearrange("p (h d) -> p h d", h=2)
            for tt in range(4):
                for ch in range(KCH):
                    nc.tensor.matmul(
                        vps[:, tt * P:(tt + 1) * P],
                        lhsT=xt[ch][:, tt * P:(tt + 1) * P],
                        rhs=wv_sb[:, ch * P:(ch + 1) * P],
                        start=(ch == 0), stop=(ch == KCH - 1))
                dst = vas[s][:, tt * VAW:(tt + 1) * VAW].rearrange(
                    "p (h x) -> p h x", x=2 * D)[:, :, 0:D]
                src = vps[:, tt * P:(tt + 1) * P].rearrange(
                    "p (h d) -> p h d", h=2)
                nc.vector.tensor_add(dst, src, bvb2)
                yield

        # Attention is split into two decoupled emission streams per q-tile:
        #   A (sexp_units): S matmuls -> exp -> mask, paced by poolS/ptpool
        #   B (av_units):   AV accumulation -> normalize -> proj -> out
        # A-streams of several q-tiles interleave freely (they never touch
        # pys), while B-streams stay strictly sequential, so a new tile's
        # S/exp runs ahead without the AV head-of-line deadlock on pys banks.
        pts = {}
        aprog = {}
        bprog = {}

        def sexp_units(b, qi):
            nkb = 4 * qi + 4   # k blocks of 128 covering [0, (qi+1)*512)
            sq = 4 * b + qi
            key = (b, qi)

            def emit_S(kb):
                c0 = max(0, kb * KBL - qi * QTL)
                sk, kc = 4 * b + kb // 4, (kb % 4) * KBL
                ps = poolS.tile([P, 2 * QTL], F32, tag="s", name="ps")
                for h in range(2):
                    nc.tensor.matmul(
                        ps[:, h * QTL + c0:(h + 1) * QTL],
                        lhsT=kTs[sk][64 * h:64 * h + 64, kc:kc + KBL],
                        rhs=qTs[sq][64 * h:64 * h + 64, c0:QTL],
                        start=True, stop=True)
                return ps, c0

            cur = emit_S(0)
            for kb in range(nkb):
                ps, c0 = cur
                if kb + 1 < nkb:
                    cur = emit_S(kb + 1)
                pt = ptpool.tile([P, 2 * QTL], F16, tag="pT", name="pt")
                if c0 == 0:
                    nc.scalar.activation(pt[:], ps[:], AF.Exp, scale=0.125)
                else:
                    src = ps[:].rearrange("p (h q) -> p h q", h=2)[:, :, c0:]
                    dst = pt[:].rearrange("p (h q) -> p h q", h=2)[:, :, c0:]
                    nc.scalar.activation(dst, src, AF.Exp, scale=0.125)
                if kb * KBL >= qi * QTL:  # diagonal block: causal triangle
                    sl = pt[:].rearrange("p (h q) -> p h q",
                                         h=2)[:, :, c0:c0 + P]
                    m3 = mask_tri[:].rearrange(
                        "p (u f) -> p u f", u=1).broadcast_to([P, 2, P])
                    nc.vector.tensor_mul(sl, sl, m3)
                pts[(b, qi, kb)] = (pt, c0)
                aprog[key] += 1
                yield

        def av_units(b, qi):
            nkb = 4 * qi + 4
            sq = 4 * b + qi
            key = (b, qi)
            # the last q-tile takes its accumulators from poolQ (idle once
            # QKV is done) so its AVs overlap the previous q-tile's tail
            ypool, ytag = (poolQ, "q") if sq == NS - 1 else (poolY, "y")
            pys = [ypool.tile([P, QTL], F32, tag=ytag, name=f"py{h}")
                   for h in range(2)]
            for kb in range(nkb):
                pt, c0 = pts.pop((b, qi, kb))
                vo = (kb % 4) * VAW
                sk = 4 * b + kb // 4
                for h in range(2):
                    nc.tensor.matmul(
                        pys[h][:, c0:QTL],
                        lhsT=vas[sk][:, vo + 2 * D * h:vo + 2 * D * h + 2 * D],
                        rhs=pt[:, h * QTL + c0:(h + 1) * QTL],
                        start=(kb == 0), stop=(kb == nkb - 1))
                bprog[key] += 1
                yield
            # normalize: y * (1/Z): broadcast Z rows to 128 partitions with a
            # rank-1 matmul, then reciprocal + scale in 128-col chunks
            nc.vector.tensor_copy(zrowp[0:1, :], pys[0][D:D + 1, :])
            nc.vector.tensor_copy(zrowp[32:33, :], pys[1][D:D + 1, :])
            pz = poolS.tile([P, QTL], F32, tag="s", name="pz")
            nc.tensor.matmul(pz[:], lhsT=e33[:], rhs=zrowp[:],
                             start=True, stop=True)
            yt = yts[sq]
            yield
            for tt4 in range(4):
                cs = slice(tt4 * P, (tt4 + 1) * P)
                zbr = zbpool.tile([P, P], F32, tag="zb", name="zbr")
                nc.vector.reciprocal(zbr[:], pz[:, cs])
                for h in range(2):
                    nc.vector.tensor_mul(
                        yt[64 * h:64 * h + D, cs],
                        pys[h][0:D, cs],
                        zbr[64 * h:64 * h + D, :],
                    )
                if tt4 % 2 == 1:
                    yield
            for tt4 in range(4):
                cs = slice(tt4 * P, (tt4 + 1) * P)
                pos = poolS.tile([P, E], F32, tag="s", name="po")
                for oc in range(2):
                    nc.tensor.matmul(
                        pos[:, oc * 512:(oc + 1) * 512],
                        lhsT=yt[:, cs],
                        rhs=wp_sb[:, oc * 512:(oc + 1) * 512],
                        start=True, stop=True)
                ti = (b * T + qi * QTL) // P + tt4
                ot = opool.tile([P, E], F32, tag="ot", name="ot")
                if tt4 % 2 == 0:
                    nc.vector.tensor_copy(ot[:], pos[:])
                else:
                    nc.scalar.activation(ot[:], pos[:], AF.Copy)
                nc.gpsimd.dma_start(outr[ti], ot[:])
                yield

        # ---- software-pipelined emission driver ----
        # Invariant: A never emits more than 4 unconsumed blocks (8 pt bufs
        # minus slack), so an exp can never block on a pt slot whose AV sits
        # behind it - the resource cycle that deadlocks poolS/pz otherwise.
        from collections import deque
        apending = []          # list of (gen, key)
        bqueue = deque()
        arr = [0]
        A_BOUND = 8

        def a_ahead():
            return sum(aprog.values()) - sum(bprog.values())

        def _advance(g):
            try:
                next(g)
                return True
            except StopIteration:
                return False

        def pump_a(n, key=None):
            pumped = 0
            while pumped < n and apending:
                if key is not None:
                    cand = [e for e in apending if e[1] == key]
                    if not cand:
                        return pumped
                    e = cand[0]
                else:
                    e = apending[arr[0] % len(apending)]
                if _advance(e[0]):
                    pumped += 1
                    arr[0] += 1
                else:
                    apending.remove(e)
            return pumped

        def pump_b(n):
            # only the head B-stream advances; its AV(kb) may only emit once
            # A emitted exp(kb) (pts handoff)
            pumped = 0
            while pumped < n and bqueue:
                g, key, nkb = bqueue[0]
                if bprog[key] < nkb and bprog[key] >= aprog[key]:
                    # head starved of its A: advance that A-gen directly
                    if a_ahead() >= A_BOUND + 2 or pump_a(1, key=key) == 0:
                        return pumped
                    continue
                if _advance(g):
                    pumped += 1
                else:
                    bqueue.popleft()
            return pumped

        def pump_mix(na, nb):
            pump_b(nb)
            while na > 0 and apending and a_ahead() < A_BOUND:
                if pump_a(1) == 0:
                    break
                na -= 1
                pump_b(1)

        QU = 14  # units per qkv super (1 issue + 8 chunks + 1 drain + 4 v)
        for s in range(NS):
            # pace the A-backlog over ~2 supers so exp work accumulates
            # toward the exp-paced back half; keep B close behind A
            start_backlog = sum(n2 - aprog[k2] for _, k2, n2 in bqueue)
            b, qi = divmod(s, NQT)
            done, k = 0, 0
            for u in qkv_units(s):
                k += 1
                want = (start_backlog * k) // QU
                if want > done:
                    pump_mix(want - done, 1)
                    done = want
                pump_b(2)
            key = (b, qi)
            aprog[key] = 0
            bprog[key] = 0
            apending.append((sexp_units(b, qi), key))
            bqueue.append((av_units(b, qi), key, 4 * qi + 4))
        while apending or bqueue:
            got = pump_b(2)
            while apending and a_ahead() < A_BOUND and pump_a(1):
                got += 1
                pump_b(1)
            if got == 0:
                if pump_b(1) == 0 and (not apending or pump_a(1) == 0):
                    break


_NC_CACHE = None


def _build():
    global _NC_CACHE
    if _NC_CACHE is None:
        nc = bacc.Bacc("TRN2", target_bir_lowering=False, debug=False)
        with tile.TileContext(nc) as tc:
            with ExitStack() as ctx:
                _emit(nc, tc, ctx)
        nc.compile()
        _NC_CACHE = nc
    return _NC_CACHE


def make_in_maps(x, w_qkv, b_qkv, w_proj):
    x2 = np.asarray(x, dtype=np.float32).reshape(TOK, E).astype(np.float16)
    xT = np.ascontiguousarray(x2.T)  # [E, TOK] feature-major
    w_qkv = np.asarray(w_qkv, dtype=np.float32)
    b_qkv = np.asarray(b_qkv, dtype=np.float32)
    w_proj = np.asarray(w_proj, dtype=np.float32)
    def cm(w):  # [E, P] slice -> [P, E] chunk-major fp16
        return np.ascontiguousarray(
            w.astype(np.float16).reshape(KCH, P, P).transpose(1, 0, 2)
            .reshape(P, E))

    in_maps = []
    for c in range(N_CORES):
        lo = P * c
        in_maps.append({
            "xT": xT,
            "wq": cm(w_qkv[:, lo:lo + P]),
            "wk": cm(w_qkv[:, E + lo:E + lo + P]),
            "wv": cm(w_qkv[:, 2 * E + lo:2 * E + lo + P]),
            "bqk": np.ascontiguousarray(
                np.stack([b_qkv[lo:lo + P],
                          b_qkv[E + lo:E + lo + P]], axis=1)
                .astype(np.float32)),
            "bv": np.ascontiguousarray(
                b_qkv[2 * E + lo:2 * E + lo + P].astype(np.float16)
                .reshape(1, P)),
            "wp": np.ascontiguousarray(w_proj[lo:lo + P, :].astype(np.float16)),
        })
    return in_maps


def run_sharded(inputs, trace=False, **kw):
    nc = _build()
    in_maps = make_in_maps(inputs["x"], inputs["w_qkv"], inputs["b_qkv"],
                           inputs["w_proj"])
    res = run_bass_kernel_spmd(nc, in_maps, list(range(N_CORES)), trace=trace,
                               **kw)
    partial = np.zeros((TOK, E), dtype=np.float32)
    for i in range(N_CORES):
        partial += res.results[i]["out"]
    out = partial + np.asarray(inputs["b_proj"], dtype=np.float32)[None, :]
    return out.reshape(B, T, E), res


def kernel(**inputs) -> np.ndarray:
    out, _ = run_sharded(inputs, trace=False)
    return out


# revision 74
# speedup vs baseline: 1.2147x; 1.2054x over previous
"""Trainium2 Bass kernel: 16-head causal self-attention block (QKV proj ->
causal MHA -> output proj), tensor-parallel over heads across 8 NeuronCores.

Contract: kernel(**inputs) takes FULL unsharded inputs
  x      [2, 2048, 1024] f32
  w_qkv  [1024, 3072] f32, b_qkv [3072] f32
  w_proj [1024, 1024] f32, b_proj [1024] f32
and returns the FULL output [2, 2048, 1024] f32.

Sharding: head-parallel. Core c owns global heads (2c, 2c+1):
  - column-parallel QKV (each core takes its 128 q/k/v feature columns)
  - full causal attention for its 2 heads (both batches)
  - row-parallel output projection -> partial [4096, 1024] sums
  - host reduces the 8 partials and adds b_proj.

v2 dataflow (vs v1): software-pipelined emission interleaves QKV of
super-tile s with attention of super s-1 so the PE never starves while
ScalarE runs softmax exp (ScalarE does ONLY exp + a few psum drains; no
Ln -> no act-table thrash). q/k/P/v all fp16 on the PE. v is computed in
natural [token, feat] orientation directly (xT stationary), killing the
PE transposes; v bias folds in at the PSUM drain via a precomputed
broadcast tile. x is pre-transposed on the host so all x loads are plain
contiguous DMAs. 1/Z via DVE reciprocal + rank-1 broadcast matmul.
"""

import numpy as np
from contextlib import ExitStack

import concourse.bass as bass
import concourse.tile as tile
from concourse import bacc, mybir
from concourse.bass_utils import run_bass_kernel_spmd
from concourse.masks import make_upper_triangular

F32 = mybir.dt.float32
F32R = mybir.dt.float32r
F16 = mybir.dt.float16
AF = mybir.ActivationFunctionType

N_CORES = 8
B, T, E, H, D = 2, 2048, 1024, 16, 64
TOK = B * T          # 4096 tokens
P = 128              # partitions
SUPER = 512          # tokens per QKV super-tile
NS = TOK // SUPER    # 8 super-tiles
KCH = E // P         # 8 contraction chunks
QTL = 512            # attention q-tile width
NQT = T // QTL       # 4 q-tiles per batch
KBL = 128            # attention k-block height
VAW = 256            # v_aug cols per token tile: 2 heads x (64 v + 64 ones);
                     # the 64 replicated ones-columns make the AV matmul emit
                     # the softmax denominator Z broadcast on partitions 64-127


def r(ap):
    return ap.bitcast(F32R)


def _emit(nc, tc, ctx):
    # weights arrive host-relayouted: [P, E] chunk-major, one contiguous DMA
    xT_h = nc.declare_dram_parameter("xT", [E, TOK], F16, isOutput=False)
    wq_h = nc.declare_dram_parameter("wq", [P, E], F16, isOutput=False)
    wk_h = nc.declare_dram_parameter("wk", [P, E], F16, isOutput=False)
    wv_h = nc.declare_dram_parameter("wv", [P, E], F16, isOutput=False)
    bqk_h = nc.declare_dram_parameter("bqk", [P, 2], F32, isOutput=False)
    bv_h = nc.declare_dram_parameter("bv", [1, P], F16, isOutput=False)
    wp_h = nc.declare_dram_parameter("wp", [P, E], F16, isOutput=False)
    out_h = nc.declare_dram_parameter("out", [TOK, E], F32, isOutput=True)

    outr = out_h[:].rearrange("(n p) e -> n p e", p=P)  # [32, 128, 1024]

    # ---------------- persistent tiles ----------------
    const = ctx.enter_context(tc.tile_pool(name="const", bufs=1))
    mask_tri = const.tile([P, P], F16)  # mask[p, f] = 1.0 iff p <= f
    make_upper_triangular(nc, mask_tri[:], val=1.0, diag=True)
    ones1h = const.tile([1, P], F16)
    nc.vector.memset(ones1h[:], 1.0)
    e33 = const.tile([33, P], F16)
    nc.vector.memset(e33[:], 0.0)
    nc.vector.memset(e33[0:1, 0:D], 1.0)
    nc.vector.memset(e33[32:33, D:2 * D], 1.0)

    # weight/bias loads go on the Activation queue (idle at startup) so the
    # x loads on the sync queue aren't delayed behind them
    bqk_sb = const.tile([P, 2], F32)
    bvr = const.tile([1, P], F16)
    nc.scalar.dma_start(bqk_sb[:], bqk_h[:])
    nc.scalar.dma_start(bvr[:], bv_h[:])
    bq_sb = bqk_sb[:, 0:1]
    bk_sb = bqk_sb[:, 1:2]
    wq_sb = const.tile([P, E], F16)
    wk_sb = const.tile([P, E], F16)
    wv_sb = const.tile([P, E], F16)
    wp_sb = const.tile([P, E], F16)
    for wsb, wh in ((wq_sb, wq_h), (wk_sb, wk_h), (wv_sb, wv_h),
                    (wp_sb, wp_h)):
        nc.scalar.dma_start(wsb[:], wh[:])
    bvb = const.tile([P, P], F32)  # v bias broadcast to all 128 partitions

    persist = ctx.enter_context(tc.tile_pool(name="persist", bufs=1))
    zrowp = persist.tile([33, QTL], F16)
    nc.vector.memset(zrowp[:], 0.0)

    with ExitStack() as ph:
        xpool = ph.enter_context(tc.tile_pool(name="xp", bufs=4))
        ptpool = ph.enter_context(tc.tile_pool(name="pTp", bufs=12))
        zbpool = ph.enter_context(tc.tile_pool(name="zbp", bufs=2))
        opool = ph.enter_context(tc.tile_pool(name="op", bufs=4))
        poolQ = ph.enter_context(tc.tile_pool(name="poolQ", bufs=2, space="PSUM"))
        poolS = ph.enter_context(tc.tile_pool(name="poolS", bufs=2, space="PSUM"))
        poolY = ph.enter_context(tc.tile_pool(name="poolY", bufs=2, space="PSUM"))

        # v bias broadcast: bvb[p, j] = bv[j] via rank-1 matmul of ones x bv
        pb = poolQ.tile([P, P], F32, tag="q", name="pb")
        nc.tensor.matmul(pb[:], lhsT=ones1h[:], rhs=bvr[:], start=True,
                         stop=True)
        nc.vector.tensor_copy(bvb[:], pb[:])

        yts = [persist.tile([P, QTL], F16, tag=f"yt{n}", name="yt")
               for n in range(NS)]
        projq = []
        qTs, kTs, vas = [], [], []
        for s in range(NS):
            qTs.append(persist.tile([P, SUPER], F16, tag=f"qT{s}", name="qTt"))
            kTs.append(persist.tile([P, SUPER], F16, tag=f"kT{s}", name="kTt"))
            vat = persist.tile([P, 4 * VAW], F16, tag=f"va{s}", name="vat")
            vas.append(vat)
            # whole tile starts as ones; the per-super v drains overwrite the
            # v columns, leaving the replicated ones-columns (64-127 of each
            # head group) that make the AV matmul emit Z on partitions 64-127
            nc.gpsimd.memset(vat[:], 1.0)

        xtiles = {}
        xT_cm = xT_h[:].rearrange("(c p) tok -> p c tok", p=P)  # [128,8,4096]

        def issue_x(s):
            # one strided DMA per super-tile; chunk ch lands contiguous at
            # cols [ch*512, (ch+1)*512)
            xt = xpool.tile([P, KCH * SUPER], F16, tag="xT", name="xTt")
            nc.sync.dma_start(
                xt[:].rearrange("p (c t) -> p c t", c=KCH),
                xT_cm[:, :, s * SUPER:(s + 1) * SUPER])
            xtiles[s] = [xt[:, ch * SUPER:(ch + 1) * SUPER]
                         for ch in range(KCH)]

        issue_x(0)
        issue_x(1)
        issue_x(2)

        def qkv_units(s):
            xt = xtiles[s]
            if s + 3 < NS:
                issue_x(s + 3)
            yield
            pfq = poolQ.tile([P, SUPER], F32, tag="q", name="pfq")
            pfk = poolQ.tile([P, SUPER], F32, tag="q", name="pfk")
            for ch in range(KCH):
                nc.tensor.matmul(
                    pfq[:], lhsT=wq_sb[:, ch * P:(ch + 1) * P],
                    rhs=xt[ch][:], start=(ch == 0), stop=(ch == KCH - 1))
                nc.tensor.matmul(
                    pfk[:], lhsT=wk_sb[:, ch * P:(ch + 1) * P],
                    rhs=xt[ch][:], start=(ch == 0), stop=(ch == KCH - 1))
                yield
            nc.vector.tensor_scalar_add(qTs[s][:], pfq[:], bq_sb)
            nc.vector.tensor_scalar_add(kTs[s][:], pfk[:], bk_sb)
            yield
            vps = poolQ.tile([P, SUPER], F32, tag="q", name="vps")
            bvb2 = bvb[:].rearrange("p (h d) -> p h d", h=2)
            for tt in range(4):
                for ch in range(KCH):
                    nc.tensor.matmul(
                        vps[:, tt * P:(tt + 1) * P],
                        lhsT=xt[ch][:, tt * P:(tt + 1) * P],
                        rhs=wv_sb[:, ch * P:(ch + 1) * P],
                        start=(ch == 0), stop=(ch == KCH - 1))
                dst = vas[s][:, tt * VAW:(tt + 1) * VAW].rearrange(
                    "p (h x) -> p h x", x=2 * D)[:, :, 0:D]
                src = vps[:, tt * P:(tt + 1) * P].rearrange(
                    "p (h d) -> p h d", h=2)
                nc.vector.tensor_add(dst, src, bvb2)
                yield

        projq = []

        def attn_units(b, qi):
            nkb = 4 * qi + 4   # k blocks of 128 covering [0, (qi+1)*512)
            sq = 4 * b + qi    # super-tile holding this q range
            # the last q-tile takes its accumulators from poolQ (idle once
            # QKV is done) so its blocks overlap the previous q-tile's tail
            ypool, ytag = (poolQ, "q") if sq == NS - 1 else (poolY, "y")
            pys = [ypool.tile([P, QTL], F32, tag=ytag, name=f"py{h}")
                   for h in range(2)]

            def emit_S(kb):
                c0 = max(0, kb * KBL - qi * QTL)
                sk, kc = 4 * b + kb // 4, (kb % 4) * KBL
                ps = poolS.tile([P, 2 * QTL], F32, tag="s", name="ps")
                for h in range(2):
                    nc.tensor.matmul(
                        ps[:, h * QTL + c0:(h + 1) * QTL],
                        lhsT=kTs[sk][64 * h:64 * h + 64, kc:kc + KBL],
                        rhs=qTs[sq][64 * h:64 * h + 64, c0:QTL],
                        start=True, stop=True)
                return ps, c0

            cur = emit_S(0)
            for kb in range(nkb):
                ps, c0 = cur
                if kb + 1 < nkb:
                    cur = emit_S(kb + 1)
                pt = ptpool.tile([P, 2 * QTL], F16, tag="pT", name="pt")
                if c0 == 0:
                    nc.scalar.activation(pt[:], ps[:], AF.Exp, scale=0.125)
                else:
                    src = ps[:].rearrange("p (h q) -> p h q", h=2)[:, :, c0:]
                    dst = pt[:].rearrange("p (h q) -> p h q", h=2)[:, :, c0:]
                    nc.scalar.activation(dst, src, AF.Exp, scale=0.125)
                if kb * KBL >= qi * QTL:  # diagonal block: causal triangle
                    sl = pt[:].rearrange("p (h q) -> p h q",
                                         h=2)[:, :, c0:c0 + P]
                    m3 = mask_tri[:].rearrange(
                        "p (u f) -> p u f", u=1).broadcast_to([P, 2, P])
                    nc.vector.tensor_mul(sl, sl, m3)
                vo = (kb % 4) * VAW
                sk = 4 * b + kb // 4
                for h in range(2):
                    nc.tensor.matmul(
                        pys[h][:, c0:QTL],
                        lhsT=vas[sk][:, vo + 2 * D * h:vo + 2 * D * h + 2 * D],
                        rhs=pt[:, h * QTL + c0:(h + 1) * QTL],
                        start=(kb == 0), stop=(kb == nkb - 1))
                yield
            # normalize: y * (1/Z): broadcast Z rows to 128 partitions with a
            # rank-1 matmul, then reciprocal + scale in 128-col chunks
            nc.vector.tensor_copy(zrowp[0:1, :], pys[0][D:D + 1, :])
            nc.vector.tensor_copy(zrowp[32:33, :], pys[1][D:D + 1, :])
            pz = poolS.tile([P, QTL], F32, tag="s", name="pz")
            nc.tensor.matmul(pz[:], lhsT=e33[:], rhs=zrowp[:],
                             start=True, stop=True)
            yt = yts[sq]
            yield
            for tt4 in range(4):
                cs = slice(tt4 * P, (tt4 + 1) * P)
                zbr = zbpool.tile([P, P], F32, tag="zb", name="zbr")
                nc.vector.reciprocal(zbr[:], pz[:, cs])
                for h in range(2):
                    nc.vector.tensor_mul(
                        yt[64 * h:64 * h + D, cs],
                        pys[h][0:D, cs],
                        zbr[64 * h:64 * h + D, :],
                    )
                if tt4 % 2 == 1:
                    yield
            if sq in (4, 5, 6):
                # deferred: dense PE work reserved for the exp-paced tail so
                # the HAM clock stays warm through the last q-tiles
                projq.append(proj_units(b, qi))
            else:
                yield from proj_units(b, qi)

        def proj_units(b, qi):
            yt = yts[4 * b + qi]
            for tt4 in range(4):
                cs = slice(tt4 * P, (tt4 + 1) * P)
                pos = poolS.tile([P, E], F32, tag="s", name="po")
                for oc in range(2):
                    nc.tensor.matmul(
                        pos[:, oc * 512:(oc + 1) * 512],
                        lhsT=yt[:, cs],
                        rhs=wp_sb[:, oc * 512:(oc + 1) * 512],
                        start=True, stop=True)
                ti = (b * T + qi * QTL) // P + tt4
                ot = opool.tile([P, E], F32, tag="ot", name="ot")
                if tt4 % 2 == 0:
                    nc.vector.tensor_copy(ot[:], pos[:])
                else:
                    nc.scalar.activation(ot[:], pos[:], AF.Copy)
                nc.gpsimd.dma_start(outr[ti], ot[:])
                yield

        # ---- software-pipelined emission driver ----
        from collections import deque
        pending = deque()
        backlog = [0]
        rr = [0]

        def pump(n):
            # round-robin across active attention generators so a finishing
            # q-tile's normalize tail interleaves with the next q-tile's
            # S blocks in every engine FIFO
            while n > 0 and pending:
                idx = rr[0] % len(pending)
                g = pending[idx]
                try:
                    next(g)
                    backlog[0] -= 1
                    n -= 1
                    rr[0] = idx + 1
                except StopIteration:
                    pending.remove(g)

        QU = 14  # units per qkv super (1 issue + 8 chunks + 1 drain + 4 v)
        for s in range(NS):
            # pace the attention backlog evenly across this super's qkv units
            # so the ScalarE exp stream never starves at a super boundary
            start_backlog = backlog[0]
            done, k = 0, 0
            for u in qkv_units(s):
                k += 1
                want = (start_backlog * k + QU - 1) // QU
                while done < want and pending:
                    pump(1)
                    done += 1
            b, qi = divmod(s, NQT)
            pending.append(attn_units(b, qi))
            backlog[0] += (4 * qi + 4) + 7
        while pending or projq:
            while projq:
                pending.append(projq.pop(0))
            pump(1)


_NC_CACHE = None


def _build():
    global _NC_CACHE
    if _NC_CACHE is None:
        nc = bacc.Bacc("TRN2", target_bir_lowering=False, debug=False)
        with tile.TileContext(nc) as tc:
            with ExitStack() as ctx:
                _emit(nc, tc, ctx)
        nc.compile()
        _NC_CACHE = nc
    return _NC_CACHE


def make_in_maps(x, w_qkv, b_qkv, w_proj):
    x2 = np.asarray(x, dtype=np.float32).reshape(TOK, E).astype(np.float16)
    xT = np.ascontiguousarray(x2.T)  # [E, TOK] feature-major
    w_qkv = np.asarray(w_qkv, dtype=np.float32)
    b_qkv = np.asarray(b_qkv, dtype=np.float32)
    w_proj = np.asarray(w_proj, dtype=np.float32)
    def cm(w):  # [E, P] slice -> [P, E] chunk-major fp16
        return np.ascontiguousarray(
            w.astype(np.float16).reshape(KCH, P, P).transpose(1, 0, 2)
            .reshape(P, E))

    in_maps = []
    for c in range(N_CORES):
        lo = P * c
        in_maps.append({
            "xT": xT,
            "wq": cm(w_qkv[:, lo:lo + P]),
            "wk": cm(w_qkv[:, E + lo:E + lo + P]),
            "wv": cm(w_qkv[:, 2 * E + lo:2 * E + lo + P]),
            "bqk": np.ascontiguousarray(
                np.stack([b_qkv[lo:lo + P],
                          b_qkv[E + lo:E + lo + P]], axis=1)
                .astype(np.float32)),
            "bv": np.ascontiguousarray(
                b_qkv[2 * E + lo:2 * E + lo + P].astype(np.float16)
                .reshape(1, P)),
            "wp": np.ascontiguousarray(w_proj[lo:lo + P, :].astype(np.float16)),
        })
    return in_maps


def run_sharded(inputs, trace=False, **kw):
    nc = _build()
    in_maps = make_in_maps(inputs["x"], inputs["w_qkv"], inputs["b_qkv"],
                           inputs["w_proj"])
    res = run_bass_kernel_spmd(nc, in_maps, list(range(N_CORES)), trace=trace,
                               **kw)
    partial = np.zeros((TOK, E), dtype=np.float32)
    for i in range(N_CORES):
        partial += res.results[i]["out"]
    out = partial + np.asarray(inputs["b_proj"], dtype=np.float32)[None, :]
    return out.reshape(B, T, E), res


def kernel(**inputs) -> np.ndarray:
    out, _ = run_sharded(inputs, trace=False)
    return out
